# revision 1
# baseline (speedup 1.0000x reference)
"""Trainium2 Bass kernel for nn_BasicTransformerBlock (cross-attention block).

Reference computation (per batch b of 16):
  q = x[b] @ Wq                        [4096, 512]
  k/v    = ctx_txt[b] @ Wk/Wv          [77, 512]
  k/v_ip = ctx_img[b] @ Wk_ip/Wv_ip    [16, 512]
  per head h (8 heads, d=64):
    sim = q_h @ k_h.T * 0.125, softmax over keys (txt / img separately)
    out_h = ts * softmax(sim_txt) @ v_txt + is * softmax(sim_img) @ v_img
  out = merge_heads(out) @ Wo + bo     [4096, 320]

Sharding: data-parallel over batch, 2 batches per core on 8 cores.

Kernel structure (per core):
  - Weights converted to bf16 on the host (PE stationary operands must be
    2-byte for full-rate matmul); x and context are also packed on the host
    into the transposed bf16 SBUF layouts the PE consumes directly.
  - Per batch: K^T and V^T projections, then the attention epilogue is
    FUSED via associativity: VW_h = V_h @ Wo_h [keys, 320] per head
    (text/img output scales folded in), so each output chunk is one PSUM
    accumulation  out = sum_h probsT_h.T @ VW_h + 1s.T @ bo  with no
    intermediate attention-output tensor or separate out-projection.
  - Streaming pipeline over 16 units (2 batches x 8 groups of 512 tokens):
    load x^T -> Q proj -> per head: QK^T, exp (scale=0.125, no
    max-subtraction: |sim|/8 is O(1) here), per-segment sums, pair-batched
    reciprocal, normalize (txt on DVE, img on gpsimd) -> one DMA-xbar
    transpose of all heads' probs -> fused out-stage -> store.
  - Keys padded: txt at partitions/cols 0:77, img at 96:112 (PE partition
    bases must be 0/32/64; zero padding makes the dead lanes inert).
  - Engine assignment (tuned empirically): psum->sbuf copies split between
    ACT and DVE (the m!=1 / j%2 splits are load-bearing); DMA rings:
    SP = xbar transposes only, ACT = HBM loads/stores, SWDGE = weights.
"""
import sys

if "/opt/trn_rl_repo" not in sys.path:
    sys.path.insert(0, "/opt/trn_rl_repo")

import ml_dtypes
import numpy as np

import concourse.bacc as bacc
import concourse.mybir as mybir
import concourse.tile as tile
from concourse.bass_utils import run_bass_kernel_spmd

F32 = mybir.dt.float32
BF16 = mybir.dt.bfloat16
AF = mybir.ActivationFunctionType
ALU = mybir.AluOpType
X_AX = mybir.AxisListType.X

N_CORES = 8
B = 16
BPC = B // N_CORES          # batches per core
N = 4096                    # tokens
QD = 320                    # query dim
CD = 1024                   # context dim
H = 8                       # heads
D = 64                      # head dim
ID = H * D                  # 512
TXT = 77                    # text keys
IMG = 16                    # image keys
IMG0 = 96                   # partition/col offset of img keys (32-aligned)
KSPAN = IMG0 + IMG          # 112
NCH = N // 128              # 32 token chunks
NG = NCH // 4               # 8 groups of 4 chunks (512 tokens per unit)
SCALE = 0.125               # 1/sqrt(64)

_NC_CACHE = None


def _build_nc():
    nc = bacc.Bacc("TRN2", target_bir_lowering=False, debug=False)

    # x pre-packed on host: x[b, p, c, k, m] = x_orig[b, 128*c+m, 128*k+p]
    x = nc.dram_tensor("x", [BPC, 128, NCH, 3, 128], BF16,
                       kind="ExternalInput").ap()
    # context pre-packed on host: ctx[b, p, k, key] = ctx_orig[b, key', 128*k+p]
    # with txt keys at 0:77, img keys at 96:112, zero padding elsewhere
    ctx = nc.dram_tensor("context", [BPC, 128, 8, 128], BF16,
                         kind="ExternalInput").ap()
    Wq = nc.dram_tensor("Wq", [QD, ID], BF16, kind="ExternalInput").ap()
    Wk = nc.dram_tensor("Wk", [CD, ID], BF16, kind="ExternalInput").ap()
    Wv = nc.dram_tensor("Wv", [CD, ID], BF16, kind="ExternalInput").ap()
    Wk_ip = nc.dram_tensor("Wk_ip", [CD, ID], BF16, kind="ExternalInput").ap()
    Wv_ip = nc.dram_tensor("Wv_ip", [CD, ID], BF16, kind="ExternalInput").ap()
    Wo = nc.dram_tensor("Wo", [ID, QD], BF16, kind="ExternalInput").ap()
    bo = nc.dram_tensor("bo", [QD], BF16, kind="ExternalInput").ap()
    tscale = nc.dram_tensor("text_scale", [1], F32, kind="ExternalInput").ap()
    iscale = nc.dram_tensor("img_scale", [1], F32, kind="ExternalInput").ap()
    out = nc.dram_tensor("out", [BPC, N, QD], F32, kind="ExternalOutput").ap()

    with tile.TileContext(nc) as tc:
        with tc.tile_pool(name="wpool", bufs=1) as wpool, \
             tc.tile_pool(name="kvpool", bufs=2) as kvpool, \
             tc.tile_pool(name="upool", bufs=6) as upool, \
             tc.tile_pool(name="appool", bufs=2) as appool, \
             tc.tile_pool(name="opool", bufs=4) as opool, \
             tc.tile_pool(name="pp", bufs=2, space="PSUM") as pp:

            # ---------------- weights (already bf16 from host) -------------
            def load_w(dram_ap, kt_count, mdim, name):
                wbf = wpool.tile([128, kt_count, mdim], BF16, name=f"w_{name}")
                nc.gpsimd.dma_start(
                    out=wbf[:],
                    in_=dram_ap.rearrange("(k p) m -> p k m", p=128))
                return wbf

            wq = wpool.tile([128, 3, ID], BF16)
            nc.scalar.dma_start(
                out=wq[:, 0:2, :],
                in_=Wq[0:256, :].rearrange("(k p) m -> p k m", p=128))
            nc.scalar.dma_start(out=wq[0:64, 2, :], in_=Wq[256:320, :])
            wk = load_w(Wk, 8, ID, "wk")
            wkip = load_w(Wk_ip, 8, ID, "wkip")
            wv = load_w(Wv, 8, ID, "wv")
            wvip = load_w(Wv_ip, 8, ID, "wvip")
            wo = load_w(Wo, 4, QD, "wo")

            bo_bf = wpool.tile([1, QD], BF16)
            nc.scalar.dma_start(out=bo_bf[:], in_=bo[None, :])
            ones1 = wpool.tile([1, 128], BF16)
            nc.gpsimd.memset(ones1[:], 1.0)

            ts_sb = wpool.tile([1, 1], F32)
            nc.scalar.dma_start(out=ts_sb[:], in_=tscale[:, None])
            is_sb = wpool.tile([1, 1], F32)
            nc.scalar.dma_start(out=is_sb[:], in_=iscale[:, None])
            ts_col = wpool.tile([128, 1], F32)
            nc.gpsimd.partition_broadcast(ts_col[:], ts_sb[:])
            is_col = wpool.tile([128, 1], F32)
            nc.gpsimd.partition_broadcast(is_col[:], is_sb[:])

            kv = []  # per-batch (kt, vw)
            for b in range(BPC):
                # ---------------- context -> K^T, V ----------------
                ctxt = kvpool.tile([128, 8, 128], BF16)
                nc.scalar.dma_start(out=ctxt[:], in_=ctx[b])

                psum_kt = pp.tile([128, 512], F32, tag="proj", bufs=2,
                                  name="psum_kt").rearrange("p (a b) -> p a b", b=128)
                for m in range(4):
                    for k in range(8):
                        nc.tensor.matmul(
                            psum_kt[:, m, 0:TXT],
                            wk[:, k, 128 * m:128 * (m + 1)],
                            ctxt[:, k, 0:TXT],
                            start=(k == 0), stop=(k == 7))
                for m in range(4):
                    for k in range(8):
                        nc.tensor.matmul(
                            psum_kt[:, m, IMG0:KSPAN],
                            wkip[:, k, 128 * m:128 * (m + 1)],
                            ctxt[:, k, IMG0:KSPAN],
                            start=(k == 0), stop=(k == 7))
                kt = kvpool.tile([128, 4, 128], BF16)
                nc.gpsimd.memset(kt[:], 0.0)
                nc.vector.tensor_copy(kt[:, :, 0:TXT], psum_kt[:, :, 0:TXT])
                nc.vector.tensor_copy(kt[:, :, IMG0:KSPAN],
                                      psum_kt[:, :, IMG0:KSPAN])

                # V^T [512 (4 m-tiles), keys], text/img scales folded in
                psum_vt = pp.tile([128, 512], F32, tag="proj", bufs=2,
                                  name="psum_vt").rearrange(
                                      "p (a c) -> p a c", c=128)
                for m in range(4):
                    for k in range(8):
                        nc.tensor.matmul(
                            psum_vt[:, m, 0:TXT],
                            wv[:, k, 128 * m:128 * (m + 1)],
                            ctxt[:, k, 0:TXT],
                            start=(k == 0), stop=(k == 7))
                for m in range(4):
                    for k in range(8):
                        nc.tensor.matmul(
                            psum_vt[:, m, IMG0:KSPAN],
                            wvip[:, k, 128 * m:128 * (m + 1)],
                            ctxt[:, k, IMG0:KSPAN],
                            start=(k == 0), stop=(k == 7))
                vt = kvpool.tile([128, 4, 128], BF16)
                nc.gpsimd.memset(vt[:], 0.0)
                nc.vector.tensor_scalar_mul(vt[:, :, 0:TXT],
                                            psum_vt[:, :, 0:TXT],
                                            ts_col[:, 0:1])
                nc.vector.tensor_scalar_mul(vt[:, :, IMG0:KSPAN],
                                            psum_vt[:, :, IMG0:KSPAN],
                                            is_col[:, 0:1])

                # VW_h = V_h @ Wo_h  [keys, 320] per head (PV and out-proj
                # then fuse: out = sum_h probsT_h.T @ VW_h)
                vw = kvpool.tile([128, 8, QD], BF16)
                for h in range(H):
                    hp, hh = h // 2, h % 2
                    psum_vw = pp.tile([128, 512], F32, tag="proj", bufs=2,
                                      name="psum_vw")
                    nc.tensor.matmul(
                        psum_vw[0:KSPAN, 0:QD],
                        vt[64 * hh:64 * (hh + 1), hp, 0:KSPAN],
                        wo[64 * hh:64 * (hh + 1), hp, :],
                        start=True, stop=True)
                    if h % 2 == 0:
                        nc.vector.tensor_copy(vw[0:KSPAN, h, :],
                                              psum_vw[0:KSPAN, 0:QD])
                    else:
                        nc.scalar.activation(vw[0:KSPAN, h, :],
                                             psum_vw[0:KSPAN, 0:QD], AF.Copy)
                kv.append((kt, vw))

            # ------------- streaming units: (batch, 512-token group) -------
            for b in range(BPC):
                kt, vw = kv[b]
                for g in range(NG):
                    # x^T already packed in DRAM: one contiguous-row load
                    xt_g = upool.tile([128, 4, 3, 128], BF16)
                    nc.scalar.dma_start(
                        out=xt_g[:], in_=x[b, :, 4 * g:4 * (g + 1), :, :])

                    # Q^T for this unit: [512 (4 m-tiles), 512 tokens]
                    qt_g = upool.tile([128, 4, 512], BF16)
                    for m in range(4):
                        psum_q = pp.tile([128, 512], F32, tag="qproj", bufs=2)
                        for ki, kp in enumerate((128, 128, 64)):
                            nc.tensor.matmul(
                                psum_q[:],
                                wq[0:kp, ki, 128 * m:128 * (m + 1)],
                                xt_g[0:kp, :, ki, :],
                                start=(ki == 0), stop=(ki == 2))
                        if m != 1:
                            nc.scalar.activation(qt_g[:, m, :], psum_q[:],
                                                 AF.Copy)
                        else:
                            nc.vector.tensor_copy(qt_g[:, m, :], psum_q[:])

                    # attention
                    probs = appool.tile([128, 8, 4, 128], BF16, tag="probs",
                                        bufs=4)
                    dsum = appool.tile([128, 8, 2, 4], F32, tag="dsum", bufs=2)
                    rsum = appool.tile([128, 8, 2, 4], F32, tag="rsum", bufs=2)
                    for hp in range(4):
                        for hh in range(2):
                            h = 2 * hp + hh
                            psum_s = pp.tile([128, 4, 128], F32, tag="sim",
                                             bufs=2, name="psum_s")
                            for c4 in range(4):
                                nc.tensor.matmul(
                                    psum_s[:, c4, 0:KSPAN],
                                    qt_g[64 * hh:64 * (hh + 1), hp,
                                         128 * c4:128 * (c4 + 1)],
                                    kt[64 * hh:64 * (hh + 1), hp, 0:KSPAN],
                                    start=True, stop=True)
                            nc.scalar.activation(
                                probs[:, h, :, 0:KSPAN],
                                psum_s[:, :, 0:KSPAN], AF.Exp, scale=SCALE)
                            nc.vector.reduce_sum(
                                out=dsum[:, h, 0, :],
                                in_=probs[:, h, :, 0:TXT], axis=X_AX)
                            nc.vector.reduce_sum(
                                out=dsum[:, h, 1, :],
                                in_=probs[:, h, :, IMG0:KSPAN], axis=X_AX)
                        h0 = 2 * hp
                        nc.vector.reciprocal(rsum[:, h0:h0 + 2, :, :],
                                             dsum[:, h0:h0 + 2, :, :])  # keep
                        for hh in range(2):
                            h = 2 * hp + hh
                            nc.vector.tensor_mul(
                                probs[:, h, :, 0:TXT],
                                probs[:, h, :, 0:TXT],
                                rsum[:, h, 0, :][:, :, None]
                                    .broadcast_to([128, 4, TXT]))
                            nc.gpsimd.tensor_mul(
                                probs[:, h, :, IMG0:KSPAN],
                                probs[:, h, :, IMG0:KSPAN],
                                rsum[:, h, 1, :][:, :, None]
                                    .broadcast_to([128, 4, IMG]))
                    probsT = appool.tile([128, 32, 128], BF16, tag="probsT",
                                         bufs=4)
                    nc.sync.dma_start(
                        out=probsT[:],
                        in_=probs.rearrange("p h c k -> p (h c k)"),
                        transpose=True)
                    # fused PV + out-proj: out_chunk = sum_h P_h @ VW_h + bo
                    out4 = opool.tile([128, 4, QD], F32)
                    for j in range(4):
                        psum_o = pp.tile([128, 512], F32, tag="pv", bufs=2,
                                         name="psum_o")
                        for h in range(H):
                            nc.tensor.matmul(
                                psum_o[:, 0:QD],
                                probsT[0:KSPAN, 4 * h + j, :],
                                vw[0:KSPAN, h, :],
                                start=(h == 0), stop=False)
                        nc.tensor.matmul(
                            psum_o[:, 0:QD], ones1[:, :], bo_bf[:, :],
                            start=False, stop=True)
                        if j % 2 == 0:
                            nc.scalar.activation(out4[:, j, :], psum_o[:, 0:QD],
                                                 AF.Copy)
                        else:
                            nc.vector.tensor_copy(out4[:, j, :], psum_o[:, 0:QD])
                    nc.scalar.dma_start(
                        out=out[b, 512 * g:512 * (g + 1), :]
                            .rearrange("(j p) d -> p j d", p=128),
                        in_=out4[:])

    nc.compile()
    return nc


def _get_nc():
    global _NC_CACHE
    if _NC_CACHE is None:
        _NC_CACHE = _build_nc()
    return _NC_CACHE


def _pack_x(x):
    # [B, N, QD] f32 -> [B, 128(p), NCH(c), 3(k), 128(m)] bf16,
    # value at [b, p, c, k, m] = x[b, 128*c+m, 128*k+p]
    xbf = np.asarray(x, np.float32).astype(ml_dtypes.bfloat16)
    xbf = xbf.reshape(B, NCH, 128, QD)                  # b, c, m, qd
    xp = np.zeros((B, NCH, 128, 384), ml_dtypes.bfloat16)
    xp[:, :, :, 0:QD] = xbf
    xp = xp.reshape(B, NCH, 128, 3, 128)                # b, c, m, k, p
    return np.ascontiguousarray(xp.transpose(0, 4, 1, 3, 2))


def _pack_ctx(context):
    # [B, 93, CD] f32 -> [B, 128(p), 8(k), 128(key)] bf16 with txt keys at
    # 0:77, img keys at 96:112, zeros elsewhere
    cbf = np.asarray(context, np.float32).astype(ml_dtypes.bfloat16)
    cbf = cbf.reshape(B, 93, 8, 128).transpose(0, 3, 2, 1)  # b, p, k, key93
    cp = np.zeros((B, 128, 8, 128), ml_dtypes.bfloat16)
    cp[:, :, :, 0:TXT] = cbf[:, :, :, 0:TXT]
    cp[:, :, :, IMG0:KSPAN] = cbf[:, :, :, TXT:93]
    return np.ascontiguousarray(cp)


def kernel(x, context, Wq, Wk, Wv, Wk_ip, Wv_ip, Wo, bo, text_scale, img_scale):
    x = _pack_x(x)
    context = _pack_ctx(context)
    bf = lambda a: np.ascontiguousarray(
        np.asarray(a, np.float32).astype(ml_dtypes.bfloat16))
    shared = {
        "Wq": bf(Wq), "Wk": bf(Wk), "Wv": bf(Wv), "Wk_ip": bf(Wk_ip),
        "Wv_ip": bf(Wv_ip), "Wo": bf(Wo), "bo": bf(bo),
        "text_scale": np.asarray(text_scale, np.float32),
        "img_scale": np.asarray(img_scale, np.float32),
    }
    nc = _get_nc()
    in_maps = []
    for c in range(N_CORES):
        m = dict(shared)
        m["x"] = x[BPC * c:BPC * (c + 1)]
        m["context"] = context[BPC * c:BPC * (c + 1)]
        in_maps.append(m)
    res = run_bass_kernel_spmd(nc, in_maps, core_ids=list(range(N_CORES)))
    return np.concatenate([res.results[c]["out"] for c in range(N_CORES)], axis=0)



# revision 2
# speedup vs baseline: 1.0146x; 1.0146x over previous
"""Trainium2 Bass kernel for nn_BasicTransformerBlock (cross-attention block).

Reference computation (per batch b of 16):
  q = x[b] @ Wq                        [4096, 512]
  k/v    = ctx_txt[b] @ Wk/Wv          [77, 512]
  k/v_ip = ctx_img[b] @ Wk_ip/Wv_ip    [16, 512]
  per head h (8 heads, d=64):
    sim = q_h @ k_h.T * 0.125, softmax over keys (txt / img separately)
    out_h = ts * softmax(sim_txt) @ v_txt + is * softmax(sim_img) @ v_img
  out = merge_heads(out) @ Wo + bo     [4096, 320]

Sharding: data-parallel over batch, 2 batches per core on 8 cores.

Kernel structure (per core), v2 (wide-instruction restructure):
  - Keys packed contiguously per head: txt at 0:77, img at 77:93, zero pad
    to 96.  Head pairs share one QK matmul: kt2 [128, hp, 192] holds head
    (2hp) keys in cols 0:96 (partitions 64:128 zeroed) and head (2hp+1)
    keys in cols 96:192 (partitions 0:64 zeroed), so lhsT can use the full
    K=128 q-tile and one N=192 matmul yields both heads' sims.
  - Sim PSUM is a 2-bank tile with chunks at 256-f32 stride (c*256+0:192),
    so one Exp activation per head-pair covers all 4 chunks via a regular
    (c, hh, j) access pattern -> 4 exp instructions per 512-token unit.
  - probs live in a packed [128, 4, 768] tile (head h at cols 96h:96h+96).
    Softmax denominators: ONE wide DVE reduce for all (chunk, head) txt
    sums + one for img; one reciprocal; normalize via a few wide
    TensorTensor ops split between DVE and Pool (broadcast rsum along the
    key axis).  Pad cols hold exp(0)=1 but multiply zeroed VW rows.
  - One DMA-xbar transpose of the packed probs -> probsT [128, 24, 128]
    whose global rows r = 96h + key ARE the packed PV contraction.
  - Fused PV + out-projection: VW_h = (scale_seg * V_h) @ Wo_h packed into
    vw [128, 6, 320] rows r = 96h + key; per chunk out = sum_t probsT_t.T
    @ vw_t — 6 full-K=128 accumulating matmuls, no per-head underpacking.
    bo is folded into VW head-0 txt rows (probs rows sum to 1), so no
    bias matmul in the stream loop.
  - PSUM->SBUF copies and normalize work are spread across ACT/DVE/Pool
    to keep every engine under the PE roofline.
"""
import sys

if "/opt/trn_rl_repo" not in sys.path:
    sys.path.insert(0, "/opt/trn_rl_repo")

import ml_dtypes
import numpy as np

import concourse.bacc as bacc
import concourse.mybir as mybir
import concourse.tile as tile
from concourse.bass_utils import run_bass_kernel_spmd

F32 = mybir.dt.float32
BF16 = mybir.dt.bfloat16
AF = mybir.ActivationFunctionType
ALU = mybir.AluOpType
X_AX = mybir.AxisListType.X

N_CORES = 8
B = 16
BPC = B // N_CORES          # batches per core
N = 4096                    # tokens
QD = 320                    # query dim
CD = 1024                   # context dim
H = 8                       # heads
D = 64                      # head dim
ID = H * D                  # 512
TXT = 77                    # text keys
IMG = 16                    # image keys
KEYS = TXT + IMG            # 93 packed keys per head
KPAD = 96                   # per-head key span (padded, 32-aligned)
NCH = N // 128              # 32 token chunks
NG = NCH // 4               # 8 groups of 4 chunks (512 tokens per unit)
SCALE = 0.125               # 1/sqrt(64)
KTOT = H * KPAD             # 768 packed (head, key) rows
KT6 = KTOT // 128           # 6 PV contraction tiles

_NC_CACHE = None


def _build_nc():
    nc = bacc.Bacc("TRN2", target_bir_lowering=False, debug=False)

    # x pre-packed on host: x[b, p, c, k, m] = x_orig[b, 128*c+m, 128*k+p]
    x = nc.dram_tensor("x", [BPC, 128, NCH, 3, 128], BF16,
                       kind="ExternalInput").ap()
    # context pre-packed on host: ctx[b, p, k, key] = ctx_orig[b, key, 128*k+p]
    # with txt keys at 0:77, img keys at 77:93, zero padding to 96
    ctx = nc.dram_tensor("context", [BPC, 128, 8, KPAD], BF16,
                         kind="ExternalInput").ap()
    Wq = nc.dram_tensor("Wq", [QD, ID], BF16, kind="ExternalInput").ap()
    Wk = nc.dram_tensor("Wk", [CD, ID], BF16, kind="ExternalInput").ap()
    Wv = nc.dram_tensor("Wv", [CD, ID], BF16, kind="ExternalInput").ap()
    Wk_ip = nc.dram_tensor("Wk_ip", [CD, ID], BF16, kind="ExternalInput").ap()
    Wv_ip = nc.dram_tensor("Wv_ip", [CD, ID], BF16, kind="ExternalInput").ap()
    Wo = nc.dram_tensor("Wo", [ID, QD], BF16, kind="ExternalInput").ap()
    bo = nc.dram_tensor("bo", [QD], BF16, kind="ExternalInput").ap()
    tscale = nc.dram_tensor("text_scale", [1], F32, kind="ExternalInput").ap()
    iscale = nc.dram_tensor("img_scale", [1], F32, kind="ExternalInput").ap()
    out = nc.dram_tensor("out", [BPC, N, QD], F32, kind="ExternalOutput").ap()

    with tile.TileContext(nc) as tc:
        with tc.tile_pool(name="wpool", bufs=1) as wpool, \
             tc.tile_pool(name="kvpool", bufs=2) as kvpool, \
             tc.tile_pool(name="upool", bufs=4) as upool, \
             tc.tile_pool(name="appool", bufs=2) as appool, \
             tc.tile_pool(name="opool", bufs=4) as opool, \
             tc.tile_pool(name="pp", bufs=2, space="PSUM") as pp:

            # ---------------- weights (already bf16 from host) -------------
            def load_w(dram_ap, kt_count, mdim, name):
                wbf = wpool.tile([128, kt_count, mdim], BF16, name=f"w_{name}")
                nc.gpsimd.dma_start(
                    out=wbf[:],
                    in_=dram_ap.rearrange("(k p) m -> p k m", p=128))
                return wbf

            wq = wpool.tile([128, 3, ID], BF16)
            nc.scalar.dma_start(
                out=wq[:, 0:2, :],
                in_=Wq[0:256, :].rearrange("(k p) m -> p k m", p=128))
            nc.scalar.dma_start(out=wq[0:64, 2, :], in_=Wq[256:320, :])
            wk = load_w(Wk, 8, ID, "wk")
            wkip = load_w(Wk_ip, 8, ID, "wkip")
            wv = load_w(Wv, 8, ID, "wv")
            wvip = load_w(Wv_ip, 8, ID, "wvip")
            wo = load_w(Wo, 4, QD, "wo")

            bo_bf = wpool.tile([1, QD], BF16)
            nc.scalar.dma_start(out=bo_bf[:], in_=bo[None, :])
            # ones over txt keys only (bias is injected via head-0 txt VW
            # rows; normalized txt probs rows sum to 1)
            ones_seg = wpool.tile([1, KPAD], BF16)
            nc.gpsimd.memset(ones_seg[:], 0.0)
            nc.gpsimd.memset(ones_seg[:, 0:TXT], 1.0)

            ts_sb = wpool.tile([1, 1], F32)
            nc.scalar.dma_start(out=ts_sb[:], in_=tscale[:, None])
            is_sb = wpool.tile([1, 1], F32)
            nc.scalar.dma_start(out=is_sb[:], in_=iscale[:, None])
            ts_col = wpool.tile([128, 1], F32)
            nc.gpsimd.partition_broadcast(ts_col[:], ts_sb[:])
            is_col = wpool.tile([128, 1], F32)
            nc.gpsimd.partition_broadcast(is_col[:], is_sb[:])

            kv = []  # per-batch (kt2, vw)
            for b in range(BPC):
                # ---------------- context -> K^T, V^T ----------------
                ctxt = kvpool.tile([128, 8, KPAD], BF16)
                nc.scalar.dma_start(out=ctxt[:], in_=ctx[b])

                psum_kt = pp.tile([128, 512], F32, tag="qproj", bufs=2,
                                  name="psum_kt").rearrange(
                                      "p (a b) -> p a b", b=128)
                for m in range(4):
                    for k in range(8):
                        nc.tensor.matmul(
                            psum_kt[:, m, 0:TXT],
                            wk[:, k, 128 * m:128 * (m + 1)],
                            ctxt[:, k, 0:TXT],
                            start=(k == 0), stop=(k == 7))
                for m in range(4):
                    for k in range(8):
                        nc.tensor.matmul(
                            psum_kt[:, m, TXT:KEYS],
                            wkip[:, k, 128 * m:128 * (m + 1)],
                            ctxt[:, k, TXT:KEYS],
                            start=(k == 0), stop=(k == 7))
                # kt2: paired-head QK rhs [128, hp, 192]; head 2hp keys in
                # cols 0:93 (partitions 64:128 zero), head 2hp+1 keys in cols
                # 96:189 (partitions 0:64 zero)
                kt2 = kvpool.tile([128, 4, 2 * KPAD], BF16)
                nc.gpsimd.memset(kt2[:], 0.0)
                nc.vector.tensor_copy(kt2[0:64, :, 0:KEYS],
                                      psum_kt[0:64, :, 0:KEYS])
                nc.scalar.activation(kt2[64:128, :, KPAD:KPAD + KEYS],
                                     psum_kt[64:128, :, 0:KEYS], AF.Copy)

                psum_vt = pp.tile([128, 512], F32, tag="qproj", bufs=2,
                                  name="psum_vt").rearrange(
                                      "p (a b) -> p a b", b=128)
                for m in range(4):
                    for k in range(8):
                        nc.tensor.matmul(
                            psum_vt[:, m, 0:TXT],
                            wv[:, k, 128 * m:128 * (m + 1)],
                            ctxt[:, k, 0:TXT],
                            start=(k == 0), stop=(k == 7))
                for m in range(4):
                    for k in range(8):
                        nc.tensor.matmul(
                            psum_vt[:, m, TXT:KEYS],
                            wvip[:, k, 128 * m:128 * (m + 1)],
                            ctxt[:, k, TXT:KEYS],
                            start=(k == 0), stop=(k == 7))
                # V^T [512 (4 m-tiles), 96 keys], text/img scales folded in;
                # pad keys 93:96 stay zero so VW rows there are zero
                vt = kvpool.tile([128, 4, KPAD], BF16)
                nc.gpsimd.memset(vt[:], 0.0)
                nc.vector.tensor_scalar_mul(vt[:, :, 0:TXT],
                                            psum_vt[:, :, 0:TXT],
                                            ts_col[:, 0:1])
                nc.vector.tensor_scalar_mul(vt[:, :, TXT:KEYS],
                                            psum_vt[:, :, TXT:KEYS],
                                            is_col[:, 0:1])

                # VW_h = V_h @ Wo_h packed at global rows 96h+key across 6
                # [128, 320] tiles (out = sum_t probsT_t.T @ vw_t later)
                vw = kvpool.tile([128, KT6, QD], BF16)
                for h in range(H):
                    hp, hh = h // 2, h % 2
                    psum_vw = pp.tile([128, 512], F32, tag="pv", bufs=2,
                                      name="psum_vw")
                    nc.tensor.matmul(
                        psum_vw[0:KPAD, 0:QD],
                        vt[64 * hh:64 * (hh + 1), hp, :],
                        wo[64 * hh:64 * (hh + 1), hp, :],
                        start=True, stop=(h != 0))
                    if h == 0:
                        # inject bo into head-0 txt rows
                        nc.tensor.matmul(
                            psum_vw[0:KPAD, 0:QD], ones_seg[:, :],
                            bo_bf[:, :], start=False, stop=True)
                    # copy psum rows 0:96 to vw global rows 96h:96h+96
                    # (at most 2 pieces, all partition bases 32-aligned)
                    r0 = KPAD * h
                    t0, p0 = r0 // 128, r0 % 128
                    n0 = min(128 - p0, KPAD)
                    eng = nc.vector.tensor_copy if h % 2 == 0 else \
                        (lambda o, i: nc.scalar.activation(o, i, AF.Copy))
                    eng(vw[p0:p0 + n0, t0, :], psum_vw[0:n0, 0:QD])
                    if n0 < KPAD:
                        eng(vw[0:KPAD - n0, t0 + 1, :],
                            psum_vw[n0:KPAD, 0:QD])
                kv.append((kt2, vw))

            # ------------- streaming units: (batch, 512-token group) -------
            for b in range(BPC):
                kt2, vw = kv[b]
                for g in range(NG):
                    # x^T already packed in DRAM: one contiguous-row load
                    xt_g = upool.tile([128, 4, 3, 128], BF16)
                    nc.scalar.dma_start(
                        out=xt_g[:], in_=x[b, :, 4 * g:4 * (g + 1), :, :])

                    # Q^T for this unit: [512 (4 m-tiles), 512 tokens]
                    qt_g = upool.tile([128, 4, 512], BF16)
                    qt_copy = (
                        lambda o, i: nc.scalar.activation(o, i, AF.Copy),
                        nc.vector.tensor_copy,
                        nc.gpsimd.tensor_copy,
                        nc.vector.tensor_copy,
                    )
                    for m in range(4):
                        psum_q = pp.tile([128, 512], F32, tag="qproj", bufs=2)
                        for ki, kp in enumerate((128, 128, 64)):
                            nc.tensor.matmul(
                                psum_q[:],
                                wq[0:kp, ki, 128 * m:128 * (m + 1)],
                                xt_g[0:kp, :, ki, :],
                                start=(ki == 0), stop=(ki == 2))
                        qt_copy[m](qt_g[:, m, :], psum_q[:])

                    # ---------------- attention ----------------
                    # probs packed [128 tok, 4 chunk, 768 (=96*h + key)]
                    probs = appool.tile([128, 4, KTOT], BF16, tag="probs",
                                        bufs=4)
                    dsum = appool.tile([128, 2, 4, H], F32, tag="dsum",
                                       bufs=2)
                    rsum = appool.tile([128, 2, 4, H], F32, tag="rsum",
                                       bufs=2)
                    for hp in range(4):
                        # 2-bank sim psum; chunk c at 256-f32 stride
                        psum_s = pp.tile([128, 1024], F32, tag="sim", bufs=2,
                                         name="psum_s").rearrange(
                                             "p (c x) -> p c x", x=256)
                        for c in range(4):
                            nc.tensor.matmul(
                                psum_s[:, c, 0:2 * KPAD],
                                qt_g[:, hp, 128 * c:128 * (c + 1)],
                                kt2[:, hp, :],
                                start=True, stop=True)
                        # one exp per head pair over all 4 chunks
                        nc.scalar.activation(
                            probs[:, :, 2 * KPAD * hp:2 * KPAD * (hp + 1)]
                                .rearrange("p c (i j) -> p c i j", j=KPAD),
                            psum_s[:, :, 0:2 * KPAD]
                                .rearrange("p c (i j) -> p c i j", j=KPAD),
                            AF.Exp, scale=SCALE)

                    pv = probs.rearrange("p c (h j) -> p c h j", j=KPAD)
                    # wide softmax denominators: one reduce for txt, one img
                    nc.vector.reduce_sum(out=dsum[:, 0], in_=pv[:, :, :, 0:TXT],
                                         axis=X_AX)
                    nc.vector.reduce_sum(out=dsum[:, 1],
                                         in_=pv[:, :, :, TXT:KEYS], axis=X_AX)
                    nc.vector.reciprocal(
                        rsum.rearrange("p a b c -> p (a b c)"),
                        dsum.rearrange("p a b c -> p (a b c)"))
                    # normalize (pad cols 93:96 stay unnormalized exp(0)=1
                    # but hit zeroed VW rows): split wide muls DVE/Pool
                    nc.vector.tensor_mul(
                        pv[:, 0:1, :, 0:TXT], pv[:, 0:1, :, 0:TXT],
                        rsum[:, 0, 0:1, :][:, :, :, None]
                            .broadcast_to([128, 1, H, TXT]))
                    nc.gpsimd.tensor_mul(
                        pv[:, 1:4, :, 0:TXT], pv[:, 1:4, :, 0:TXT],
                        rsum[:, 0, 1:4, :][:, :, :, None]
                            .broadcast_to([128, 3, H, TXT]))
                    nc.vector.tensor_mul(
                        pv[:, :, :, TXT:KEYS], pv[:, :, :, TXT:KEYS],
                        rsum[:, 1][:, :, :, None]
                            .broadcast_to([128, 4, H, IMG]))

                    probsT = appool.tile([128, 4 * KT6, 128], BF16,
                                         tag="probsT", bufs=4)
                    nc.sync.dma_start(
                        out=probsT[:],
                        in_=probs.rearrange("p c k -> p (c k)"),
                        transpose=True)
                    # fused PV + out-proj: full-K=128 accumulation over the
                    # 6 packed (head, key) tiles
                    out4 = opool.tile([128, 4, QD], F32)
                    o_copy = (
                        nc.gpsimd.tensor_copy,
                        nc.vector.tensor_copy,
                        nc.gpsimd.tensor_copy,
                        (lambda o, i: nc.scalar.activation(o, i, AF.Copy)),
                    )
                    for c in range(4):
                        psum_o = pp.tile([128, 512], F32, tag="pv", bufs=2,
                                         name="psum_o")
                        for t in range(KT6):
                            nc.tensor.matmul(
                                psum_o[:, 0:QD],
                                probsT[:, KT6 * c + t, :],
                                vw[:, t, :],
                                start=(t == 0), stop=(t == KT6 - 1))
                        o_copy[c](out4[:, c, :], psum_o[:, 0:QD])
                    nc.scalar.dma_start(
                        out=out[b, 512 * g:512 * (g + 1), :]
                            .rearrange("(j p) d -> p j d", p=128),
                        in_=out4[:])

    nc.compile()
    return nc


def _get_nc():
    global _NC_CACHE
    if _NC_CACHE is None:
        _NC_CACHE = _build_nc()
    return _NC_CACHE


def _pack_x(x):
    # [B, N, QD] f32 -> [B, 128(p), NCH(c), 3(k), 128(m)] bf16,
    # value at [b, p, c, k, m] = x[b, 128*c+m, 128*k+p]
    xbf = np.asarray(x, np.float32).astype(ml_dtypes.bfloat16)
    xbf = xbf.reshape(B, NCH, 128, QD)                  # b, c, m, qd
    xp = np.zeros((B, NCH, 128, 384), ml_dtypes.bfloat16)
    xp[:, :, :, 0:QD] = xbf
    xp = xp.reshape(B, NCH, 128, 3, 128)                # b, c, m, k, p
    return np.ascontiguousarray(xp.transpose(0, 4, 1, 3, 2))


def _pack_ctx(context):
    # [B, 93, CD] f32 -> [B, 128(p), 8(k), 96(key)] bf16 with keys packed
    # contiguously (txt 0:77, img 77:93), zeros at 93:96
    cbf = np.asarray(context, np.float32).astype(ml_dtypes.bfloat16)
    cbf = cbf.reshape(B, KEYS, 8, 128).transpose(0, 3, 2, 1)  # b, p, k, key
    cp = np.zeros((B, 128, 8, KPAD), ml_dtypes.bfloat16)
    cp[:, :, :, 0:KEYS] = cbf
    return np.ascontiguousarray(cp)


def kernel(x, context, Wq, Wk, Wv, Wk_ip, Wv_ip, Wo, bo, text_scale, img_scale):
    x = _pack_x(x)
    context = _pack_ctx(context)
    bf = lambda a: np.ascontiguousarray(
        np.asarray(a, np.float32).astype(ml_dtypes.bfloat16))
    shared = {
        "Wq": bf(Wq), "Wk": bf(Wk), "Wv": bf(Wv), "Wk_ip": bf(Wk_ip),
        "Wv_ip": bf(Wv_ip), "Wo": bf(Wo), "bo": bf(bo),
        "text_scale": np.asarray(text_scale, np.float32),
        "img_scale": np.asarray(img_scale, np.float32),
    }
    nc = _get_nc()
    in_maps = []
    for c in range(N_CORES):
        m = dict(shared)
        m["x"] = x[BPC * c:BPC * (c + 1)]
        m["context"] = context[BPC * c:BPC * (c + 1)]
        in_maps.append(m)
    res = run_bass_kernel_spmd(nc, in_maps, core_ids=list(range(N_CORES)))
    return np.concatenate([res.results[c]["out"] for c in range(N_CORES)], axis=0)


# revision 4
# speedup vs baseline: 1.1406x; 1.1242x over previous
"""Trainium2 Bass kernel for nn_BasicTransformerBlock (cross-attention block).

Reference computation (per batch b of 16):
  q = x[b] @ Wq                        [4096, 512]
  k/v    = ctx_txt[b] @ Wk/Wv          [77, 512]
  k/v_ip = ctx_img[b] @ Wk_ip/Wv_ip    [16, 512]
  per head h (8 heads, d=64):
    sim = q_h @ k_h.T * 0.125, softmax over keys (txt / img separately)
    out_h = ts * softmax(sim_txt) @ v_txt + is * softmax(sim_img) @ v_img
  out = merge_heads(out) @ Wo + bo     [4096, 320]

Sharding: data-parallel over batch, 2 batches per core on 8 cores.

Kernel structure (per core), v3 (wide instructions + software pipelining):
  - Keys packed contiguously per head: txt at 0:77, img at 77:93, zero pad
    to 96.  Head pairs share one QK matmul: kt2 [128, hp, 192] holds head
    (2hp) keys in cols 0:96 (partitions 64:128 zeroed) and head (2hp+1)
    keys in cols 96:192 (partitions 0:64 zeroed), so lhsT is the full
    K=128 q-tile and one N=192 matmul yields both heads' sims.
  - Sim PSUM is a 2-bank tile with chunks at 256-f32 stride, so ONE Exp
    activation per head-pair covers all 4 chunks via a regular (c, hh, j)
    access pattern -> 4 exp instructions per 512-token unit.
  - probs packed [128, 4, 768] (head h at cols 96h:96h+96).  Softmax sums:
    one wide DVE reduce txt + one img; one reciprocal; normalize via wide
    TensorTensor muls split DVE/Pool.  Pad cols hold exp(0)=1 but multiply
    zeroed VW rows, so they are inert.
  - One DMA-xbar transpose -> probsT [128, 24, 128] whose global rows
    r = 96h + key ARE the packed PV contraction.
  - Fused PV + out-projection: VW_h = (scale_seg * V_h) @ Wo_h packed into
    vw [128, 6, 320] rows r = 96h + key; per chunk out = sum_t probsT_t.T
    @ vw_t — 6 full-K=128 accumulating matmuls.  bo is folded into VW
    head-0 txt rows (normalized probs rows sum to 1): no bias matmuls.
  - SOFTWARE PIPELINING: PE's in-order stream per iteration i is
    [PV(i-2), Qproj(i), QK(i)], so the softmax chain of unit i (ACT exp ->
    DVE reduce/recip -> DVE/Pool normalize -> SP-ring transpose) has two
    full iterations to complete before PE needs probsT(i).  Engine streams
    are ordered by dependency age to avoid head-of-line blocking; the
    PSUM-freeing copies (qt m0/m1, out4 c0/c1) sit on ACT/DVE which
    respond fast, the later ones on Pool.
  - DMA rings: ACT = x loads, DVE = out stores, SP = xbar transposes,
    SWDGE(Pool) = weight loads.
"""
import sys

if "/opt/trn_rl_repo" not in sys.path:
    sys.path.insert(0, "/opt/trn_rl_repo")

import ml_dtypes
import numpy as np

import concourse.bacc as bacc
import concourse.mybir as mybir
import concourse.tile as tile
from concourse.bass_utils import run_bass_kernel_spmd

F32 = mybir.dt.float32
BF16 = mybir.dt.bfloat16
AF = mybir.ActivationFunctionType
ALU = mybir.AluOpType
X_AX = mybir.AxisListType.X

N_CORES = 8
B = 16
BPC = B // N_CORES          # batches per core
N = 4096                    # tokens
QD = 320                    # query dim
CD = 1024                   # context dim
H = 8                       # heads
D = 64                      # head dim
ID = H * D                  # 512
TXT = 77                    # text keys
IMG = 16                    # image keys
KEYS = TXT + IMG            # 93 packed keys per head
KPAD = 96                   # per-head key span (padded, 32-aligned)
NCH = N // 128              # 32 token chunks
NG = NCH // 4               # 8 groups of 4 chunks (512 tokens per unit)
SCALE = 0.125               # 1/sqrt(64)
KTOT = H * KPAD             # 768 packed (head, key) rows
KT6 = KTOT // 128           # 6 PV contraction tiles
UNITS = BPC * NG            # 16 streaming units
LAG = 2                     # PV pipeline lag (iterations)

_NC_CACHE = None


def _build_nc():
    nc = bacc.Bacc("TRN2", target_bir_lowering=False, debug=False)

    # x pre-packed on host: x[b, p, c, k, m] = x_orig[b, 128*c+m, 128*k+p]
    x = nc.dram_tensor("x", [BPC, 128, NCH, 3, 128], BF16,
                       kind="ExternalInput").ap()
    # context pre-packed on host: ctx[b, p, k, key] = ctx_orig[b, key, 128*k+p]
    # with txt keys at 0:77, img keys at 77:93, zero padding to 96
    ctx = nc.dram_tensor("context", [BPC, 128, 8, KPAD], BF16,
                         kind="ExternalInput").ap()
    Wq = nc.dram_tensor("Wq", [QD, ID], BF16, kind="ExternalInput").ap()
    Wk = nc.dram_tensor("Wk", [CD, ID], BF16, kind="ExternalInput").ap()
    Wv = nc.dram_tensor("Wv", [CD, ID], BF16, kind="ExternalInput").ap()
    Wk_ip = nc.dram_tensor("Wk_ip", [CD, ID], BF16, kind="ExternalInput").ap()
    Wv_ip = nc.dram_tensor("Wv_ip", [CD, ID], BF16, kind="ExternalInput").ap()
    Wo = nc.dram_tensor("Wo", [ID, QD], BF16, kind="ExternalInput").ap()
    bo = nc.dram_tensor("bo", [QD], BF16, kind="ExternalInput").ap()
    tscale = nc.dram_tensor("text_scale", [1], F32, kind="ExternalInput").ap()
    iscale = nc.dram_tensor("img_scale", [1], F32, kind="ExternalInput").ap()
    out = nc.dram_tensor("out", [BPC, N, QD], F32, kind="ExternalOutput").ap()

    act_copy = lambda o, i: nc.scalar.activation(o, i, AF.Copy)

    with tile.TileContext(nc) as tc:
        with tc.tile_pool(name="wpool", bufs=1) as wpool, \
             tc.tile_pool(name="kvpool", bufs=2) as kvpool, \
             tc.tile_pool(name="xpool", bufs=3) as xpool, \
             tc.tile_pool(name="qpool", bufs=2) as qpool, \
             tc.tile_pool(name="appool", bufs=2) as appool, \
             tc.tile_pool(name="opool", bufs=3) as opool, \
             tc.tile_pool(name="pp", bufs=2, space="PSUM") as pp:

            # ---------------- weights (already bf16 from host) -------------
            def load_w(dram_ap, kt_count, mdim, name):
                wbf = wpool.tile([128, kt_count, mdim], BF16, name=f"w_{name}")
                nc.gpsimd.dma_start(
                    out=wbf[:],
                    in_=dram_ap.rearrange("(k p) m -> p k m", p=128))
                return wbf

            wq = wpool.tile([128, 3, ID], BF16)
            nc.scalar.dma_start(
                out=wq[:, 0:2, :],
                in_=Wq[0:256, :].rearrange("(k p) m -> p k m", p=128))
            nc.scalar.dma_start(out=wq[0:64, 2, :], in_=Wq[256:320, :])
            wk = load_w(Wk, 8, ID, "wk")
            wkip = load_w(Wk_ip, 8, ID, "wkip")
            wv = load_w(Wv, 8, ID, "wv")
            wvip = load_w(Wv_ip, 8, ID, "wvip")
            wo = load_w(Wo, 4, QD, "wo")

            bo_bf = wpool.tile([1, QD], BF16)
            nc.scalar.dma_start(out=bo_bf[:], in_=bo[None, :])
            # ones over txt keys only (bias is injected via head-0 txt VW
            # rows; normalized txt probs rows sum to 1)
            ones_seg = wpool.tile([1, KPAD], BF16)
            nc.gpsimd.memset(ones_seg[:], 0.0)
            nc.gpsimd.memset(ones_seg[:, 0:TXT], 1.0)

            ts_sb = wpool.tile([1, 1], F32)
            nc.scalar.dma_start(out=ts_sb[:], in_=tscale[:, None])
            is_sb = wpool.tile([1, 1], F32)
            nc.scalar.dma_start(out=is_sb[:], in_=iscale[:, None])
            ts_col = wpool.tile([128, 1], F32)
            nc.gpsimd.partition_broadcast(ts_col[:], ts_sb[:])
            is_col = wpool.tile([128, 1], F32)
            nc.gpsimd.partition_broadcast(is_col[:], is_sb[:])

            # prefetch x for the first two units
            xts = {}
            for i in range(min(2, UNITS)):
                b, g = divmod(i, NG)
                xts[i] = xpool.tile([128, 4, 3, 128], BF16, name="xt")
                nc.scalar.dma_start(
                    out=xts[i][:], in_=x[b, :, 4 * g:4 * (g + 1), :, :])

            kv = []  # per-batch (kt2, vw)
            for b in range(BPC):
                # ---------------- context -> K^T, V^T ----------------
                ctxt = kvpool.tile([128, 8, KPAD], BF16)
                nc.scalar.dma_start(out=ctxt[:], in_=ctx[b])

                psum_kt = pp.tile([128, 512], F32, tag="qproj", bufs=2,
                                  name="psum_kt").rearrange(
                                      "p (a b) -> p a b", b=128)
                for m in range(4):
                    for k in range(8):
                        nc.tensor.matmul(
                            psum_kt[:, m, 0:TXT],
                            wk[:, k, 128 * m:128 * (m + 1)],
                            ctxt[:, k, 0:TXT],
                            start=(k == 0), stop=(k == 7))
                for m in range(4):
                    for k in range(8):
                        nc.tensor.matmul(
                            psum_kt[:, m, TXT:KEYS],
                            wkip[:, k, 128 * m:128 * (m + 1)],
                            ctxt[:, k, TXT:KEYS],
                            start=(k == 0), stop=(k == 7))
                # kt2: paired-head QK rhs [128, hp, 192]
                kt2 = kvpool.tile([128, 4, 2 * KPAD], BF16)
                nc.gpsimd.memset(kt2[:], 0.0)
                nc.vector.tensor_copy(kt2[0:64, :, 0:KEYS],
                                      psum_kt[0:64, :, 0:KEYS])
                act_copy(kt2[64:128, :, KPAD:KPAD + KEYS],
                         psum_kt[64:128, :, 0:KEYS])

                psum_vt = pp.tile([128, 512], F32, tag="qproj", bufs=2,
                                  name="psum_vt").rearrange(
                                      "p (a b) -> p a b", b=128)
                for m in range(4):
                    for k in range(8):
                        nc.tensor.matmul(
                            psum_vt[:, m, 0:TXT],
                            wv[:, k, 128 * m:128 * (m + 1)],
                            ctxt[:, k, 0:TXT],
                            start=(k == 0), stop=(k == 7))
                for m in range(4):
                    for k in range(8):
                        nc.tensor.matmul(
                            psum_vt[:, m, TXT:KEYS],
                            wvip[:, k, 128 * m:128 * (m + 1)],
                            ctxt[:, k, TXT:KEYS],
                            start=(k == 0), stop=(k == 7))
                # V^T [512 (4 m-tiles), 96 keys], text/img scales folded in;
                # pad keys 93:96 stay zero so VW rows there are zero
                vt = kvpool.tile([128, 4, KPAD], BF16)
                nc.gpsimd.memset(vt[:], 0.0)
                nc.vector.tensor_scalar_mul(vt[:, :, 0:TXT],
                                            psum_vt[:, :, 0:TXT],
                                            ts_col[:, 0:1])
                nc.vector.tensor_scalar_mul(vt[:, :, TXT:KEYS],
                                            psum_vt[:, :, TXT:KEYS],
                                            is_col[:, 0:1])

                # VW_h = V_h @ Wo_h packed at global rows 96h+key across 6
                # [128, 320] tiles
                vw = kvpool.tile([128, KT6, QD], BF16)
                for h in range(H):
                    hp, hh = h // 2, h % 2
                    psum_vw = pp.tile([128, 512], F32, tag="pv", bufs=2,
                                      name="psum_vw")
                    nc.tensor.matmul(
                        psum_vw[0:KPAD, 0:QD],
                        vt[64 * hh:64 * (hh + 1), hp, :],
                        wo[64 * hh:64 * (hh + 1), hp, :],
                        start=True, stop=(h != 0))
                    if h == 0:
                        # inject bo into head-0 txt rows
                        nc.tensor.matmul(
                            psum_vw[0:KPAD, 0:QD], ones_seg[:, :],
                            bo_bf[:, :], start=False, stop=True)
                    # copy psum rows 0:96 to vw global rows 96h:96h+96
                    # (at most 2 pieces, all partition bases 32-aligned)
                    r0 = KPAD * h
                    t0, p0 = r0 // 128, r0 % 128
                    n0 = min(128 - p0, KPAD)
                    eng = nc.vector.tensor_copy if h % 2 == 0 else act_copy
                    eng(vw[p0:p0 + n0, t0, :], psum_vw[0:n0, 0:QD])
                    if n0 < KPAD:
                        eng(vw[0:KPAD - n0, t0 + 1, :],
                            psum_vw[n0:KPAD, 0:QD])
                kv.append((kt2, vw))

            # ------- software-pipelined streaming loop over 16 units -------
            stages = {}  # unit index -> dict of live tiles
            for i in range(UNITS + LAG):
                # ---- PE first: PV + out-proj for unit i-LAG (oldest deps)
                if i >= LAG:
                    st = stages.pop(i - LAG)
                    bb, gg = divmod(i - LAG, NG)
                    _, vw_u = kv[bb]
                    probsT = st["probsT"]
                    out4 = opool.tile([128, 4, QD], F32)
                    o_copy = (act_copy, nc.vector.tensor_copy,
                              nc.gpsimd.tensor_copy, nc.gpsimd.tensor_copy)
                    for c in range(4):
                        psum_o = pp.tile([128, 512], F32, tag="pv", bufs=2,
                                         name="psum_o")
                        for t in range(KT6):
                            nc.tensor.matmul(
                                psum_o[:, 0:QD],
                                probsT[:, KT6 * c + t, :],
                                vw_u[:, t, :],
                                start=(t == 0), stop=(t == KT6 - 1))
                        o_copy[c](out4[:, c, :], psum_o[:, 0:QD])
                    # store on the SP DMA ring (ACT ring carries x loads)
                    nc.sync.dma_start(
                        out=out[bb, 512 * gg:512 * (gg + 1), :]
                            .rearrange("(j p) d -> p j d", p=128),
                        in_=out4[:])

                if i >= UNITS:
                    continue
                b, g = divmod(i, NG)
                kt2, _ = kv[b]

                # prefetch x for unit i+1 (ACT DMA ring)
                if i + 1 < UNITS:
                    bn, gn = divmod(i + 1, NG)
                    xts[i + 1] = xpool.tile([128, 4, 3, 128], BF16, name="xt")
                    nc.scalar.dma_start(
                        out=xts[i + 1][:],
                        in_=x[bn, :, 4 * gn:4 * (gn + 1), :, :])

                # ---- Q projection for unit i
                xt_g = xts.pop(i)
                qt_g = qpool.tile([128, 4, 512], BF16)
                qt_copy = (act_copy, nc.vector.tensor_copy,
                           nc.gpsimd.tensor_copy, act_copy)
                for m in range(4):
                    psum_q = pp.tile([128, 512], F32, tag="qproj", bufs=2)
                    for ki, kp in enumerate((128, 128, 64)):
                        nc.tensor.matmul(
                            psum_q[:],
                            wq[0:kp, ki, 128 * m:128 * (m + 1)],
                            xt_g[0:kp, :, ki, :],
                            start=(ki == 0), stop=(ki == 2))
                    qt_copy[m](qt_g[:, m, :], psum_q[:])

                # ---- QK^T + exp for unit i
                probs = appool.tile([128, 4, KTOT], BF16, tag="probs", bufs=3)
                dsum = appool.tile([128, 2, 4, H], F32, tag="dsum", bufs=2)
                rsum = appool.tile([128, 2, 4, H], F32, tag="rsum", bufs=2)
                for hp in range(4):
                    psum_s = pp.tile([128, 1024], F32, tag="sim", bufs=2,
                                     name="psum_s").rearrange(
                                         "p (c x) -> p c x", x=256)
                    for c in range(4):
                        nc.tensor.matmul(
                            psum_s[:, c, 0:2 * KPAD],
                            qt_g[:, hp, 128 * c:128 * (c + 1)],
                            kt2[:, hp, :],
                            start=True, stop=True)
                    # one exp per head pair over all 4 chunks
                    nc.scalar.activation(
                        probs[:, :, 2 * KPAD * hp:2 * KPAD * (hp + 1)]
                            .rearrange("p c (i j) -> p c i j", j=KPAD),
                        psum_s[:, :, 0:2 * KPAD]
                            .rearrange("p c (i j) -> p c i j", j=KPAD),
                        AF.Exp, scale=SCALE)

                pv = probs.rearrange("p c (h j) -> p c h j", j=KPAD)
                # wide softmax denominators + reciprocal (DVE)
                nc.vector.reduce_sum(out=dsum[:, 0], in_=pv[:, :, :, 0:TXT],
                                     axis=X_AX)
                nc.vector.reduce_sum(out=dsum[:, 1],
                                     in_=pv[:, :, :, TXT:KEYS], axis=X_AX)
                nc.vector.reciprocal(
                    rsum.rearrange("p a b c -> p (a b c)"),
                    dsum.rearrange("p a b c -> p (a b c)"))
                # normalize: txt chunks 0-1 on DVE, 2-3 + img on Pool
                nc.vector.tensor_mul(
                    pv[:, 0:2, :, 0:TXT], pv[:, 0:2, :, 0:TXT],
                    rsum[:, 0, 0:2, :][:, :, :, None]
                        .broadcast_to([128, 2, H, TXT]))
                nc.gpsimd.tensor_mul(
                    pv[:, 2:4, :, 0:TXT], pv[:, 2:4, :, 0:TXT],
                    rsum[:, 0, 2:4, :][:, :, :, None]
                        .broadcast_to([128, 2, H, TXT]))
                nc.gpsimd.tensor_mul(
                    pv[:, :, :, TXT:KEYS], pv[:, :, :, TXT:KEYS],
                    rsum[:, 1][:, :, :, None]
                        .broadcast_to([128, 4, H, IMG]))

                # transpose on the SP ring; consumed by PV in iteration i+2
                probsT = appool.tile([128, 4 * KT6, 128], BF16,
                                     tag="probsT", bufs=4)
                nc.sync.dma_start(
                    out=probsT[:],
                    in_=probs.rearrange("p c k -> p (c k)"),
                    transpose=True)
                stages[i] = {"probsT": probsT}

    nc.compile()
    return nc


def _get_nc():
    global _NC_CACHE
    if _NC_CACHE is None:
        _NC_CACHE = _build_nc()
    return _NC_CACHE


def _pack_x(x):
    # [B, N, QD] f32 -> [B, 128(p), NCH(c), 3(k), 128(m)] bf16,
    # value at [b, p, c, k, m] = x[b, 128*c+m, 128*k+p]
    xbf = np.asarray(x, np.float32).astype(ml_dtypes.bfloat16)
    xbf = xbf.reshape(B, NCH, 128, QD)                  # b, c, m, qd
    xp = np.zeros((B, NCH, 128, 384), ml_dtypes.bfloat16)
    xp[:, :, :, 0:QD] = xbf
    xp = xp.reshape(B, NCH, 128, 3, 128)                # b, c, m, k, p
    return np.ascontiguousarray(xp.transpose(0, 4, 1, 3, 2))


def _pack_ctx(context):
    # [B, 93, CD] f32 -> [B, 128(p), 8(k), 96(key)] bf16 with keys packed
    # contiguously (txt 0:77, img 77:93), zeros at 93:96
    cbf = np.asarray(context, np.float32).astype(ml_dtypes.bfloat16)
    cbf = cbf.reshape(B, KEYS, 8, 128).transpose(0, 3, 2, 1)  # b, p, k, key
    cp = np.zeros((B, 128, 8, KPAD), ml_dtypes.bfloat16)
    cp[:, :, :, 0:KEYS] = cbf
    return np.ascontiguousarray(cp)


def kernel(x, context, Wq, Wk, Wv, Wk_ip, Wv_ip, Wo, bo, text_scale, img_scale):
    x = _pack_x(x)
    context = _pack_ctx(context)
    bf = lambda a: np.ascontiguousarray(
        np.asarray(a, np.float32).astype(ml_dtypes.bfloat16))
    shared = {
        "Wq": bf(Wq), "Wk": bf(Wk), "Wv": bf(Wv), "Wk_ip": bf(Wk_ip),
        "Wv_ip": bf(Wv_ip), "Wo": bf(Wo), "bo": bf(bo),
        "text_scale": np.asarray(text_scale, np.float32),
        "img_scale": np.asarray(img_scale, np.float32),
    }
    nc = _get_nc()
    in_maps = []
    for c in range(N_CORES):
        m = dict(shared)
        m["x"] = x[BPC * c:BPC * (c + 1)]
        m["context"] = context[BPC * c:BPC * (c + 1)]
        in_maps.append(m)
    res = run_bass_kernel_spmd(nc, in_maps, core_ids=list(range(N_CORES)))
    return np.concatenate([res.results[c]["out"] for c in range(N_CORES)], axis=0)
